# revision 22
# baseline (speedup 1.0000x reference)
"""AttentionAggregator kernel for 8 TRN2 NeuronCores.

Reference computation (per node i over M neighbors j):
    s_self  = self_feats @ a[:D]            # [N]
    s_neigh = features_neighs @ a[D:]       # [M]
    scores  = leaky_relu(s_self[:,None] + s_neigh[None,:], 0.2)
    attn    = softmax(where(mask, scores, -1e30), axis=1); attn = where(mask, attn, 0)
    out     = attn @ features_neighs        # [N, D]

Key identity used on device: with x = s_self[i] + s_neigh[j],
    exp(leaky_relu(x, 0.2)) = max(exp(x), exp(0.2 x)) = max(u_i*v_j, uh_i*vh_j)
where u = exp(s_self), uh = exp(0.2 s_self), v = exp(s_neigh), vh = exp(0.2 s_neigh).
Softmax max-subtraction is skipped (scores are O(10), exp is safe in fp32/bf16),
and masked entries are exactly zero after multiplying by the 0/1 mask, so
    out[i] = (p @ F)[i] / rowsum(p)[i],   p = mask * max(u v, uh vh).

Per node tile [128 x 4096] (all bf16 elementwise):
    ACT : A = v_bcast * u[i]                  (Copy with per-partition scale)
    DVE : B = vh_bcast * uh[i]                (tensor_scalar, 4x mode)
    DVE : C = max(B, A)                       (tensor_tensor, 2x mode)
    DMA : p = C * mask  (SWDGE accum_op=mult with int32->bf16 cast, in place)
    DMA : pT = xbar blocked transpose of p    (one InstDmaTransposeAnt)
    PE  : psum[128,129] += pT_c^T @ [F_c | 1] (32 accumulating bf16 matmuls;
                                               ones column yields rowsum)
    DVE : out = psum[:, :128] * (1/rowsum)

Sharding: rows of self_feats / neigh_matrix split across 8 cores (2048 rows
each); features_neighs and `a` replicated. No collectives.
"""

import numpy as np
from contextlib import ExitStack

N, M, D = 16384, 4096, 128
NCORES = 8
NLOC = N // NCORES          # 2048 nodes per core
P = 128                     # partitions
NT = NLOC // P              # 16 node tiles per core
NC_J = M // P               # 32 neighbor chunks

_BUILT = {}


def _build_nc(n_loc=NLOC, m=M, d=D, reps=1, fuse_mask_dma=False,
              mm_fstat=False):
    import concourse.bass as bass
    import concourse.bacc as bacc
    import concourse.tile as tile
    from concourse import mybir, masks

    f32 = mybir.dt.float32
    bf16 = mybir.dt.bfloat16
    i32 = mybir.dt.int32
    Op = mybir.AluOpType
    AF = mybir.ActivationFunctionType

    nt = n_loc // P
    nj = m // P

    nc = bacc.Bacc("TRN2", target_bir_lowering=False, debug=False,
                   num_devices=NCORES)

    self_d = nc.dram_tensor("self_feats", [n_loc, d], f32, kind="ExternalInput")
    f_d = nc.dram_tensor("features_neighs", [m, d], f32, kind="ExternalInput")
    m_d = nc.dram_tensor("neigh_matrix", [n_loc, m], i32, kind="ExternalInput")
    a_d = nc.dram_tensor("a", [2 * d, 1], f32, kind="ExternalInput")
    out_d = nc.dram_tensor("out", [n_loc, d], f32, kind="ExternalOutput")

    with tile.TileContext(nc) as tc, ExitStack() as ctx:
        const = ctx.enter_context(tc.tile_pool(name="const", bufs=1))
        maskp = ctx.enter_context(tc.tile_pool(name="maskp", bufs=6))
        pre_ctx = ExitStack()
        pre = pre_ctx.enter_context(tc.tile_pool(name="pre", bufs=4))
        psum_pre = pre_ctx.enter_context(
            tc.tile_pool(name="psum_pre", bufs=2, space="PSUM"))

        # F chunks (cast to bf16) load first: the whole precompute chain
        # hangs off this DMA, so it must win the SWDGE queue
        f_sb = const.tile([P, nj, P + 1], bf16)
        nc.gpsimd.dma_start(
            f_sb[:, :, 0:P],
            f_d.ap().rearrange("(c q) d -> q c d", q=P))
        nc.gpsimd.memset(f_sb[:].rearrange("p c q -> p (c q)")
                         [:, P::P + 1], 1.0)  # ones column per chunk

        # prefetch the first mask tiles (casting int32 -> bf16) so the HBM
        # stream saturates from t=0, overlapping the whole precompute
        mask_tiles = {}
        if not fuse_mask_dma:
            for t in range(min(6, nt * reps)):
                mt = maskp.tile([P, m], bf16, tag="mask")
                nc.gpsimd.dma_start(mt[:], m_d[(t % nt) * P:(t % nt + 1) * P, :])
                mask_tiles[t] = mt

        # ---------------- precompute ----------------
        ident = const.tile([P, P], f32)
        masks.make_identity(nc, ident[:])

        ones1 = const.tile([1, P], f32)
        nc.vector.memset(ones1[:], 1.0)

        # a as two single-partition rows: a_self, a_neigh
        a_self_row = const.tile([1, d], f32)
        a_neigh_row = const.tile([1, d], f32)
        a_flat = a_d.ap().rearrange("(one dd) o2 -> one (dd o2)", one=1)
        nc.scalar.dma_start(a_self_row[:], a_flat[:, 0:d])
        nc.scalar.dma_start(a_neigh_row[:], a_flat[:, d:2 * d])

        # broadcast a rows across all 128 partitions via PE outer product
        a_self_b = const.tile([P, d], f32)
        a_neigh_b = const.tile([P, d], f32)
        for dst, row in ((a_self_b, a_self_row), (a_neigh_b, a_neigh_row)):
            ps = psum_pre.tile([P, d], f32, tag="ps_bc")
            nc.tensor.matmul(ps[:], ones1[:], row[:])
            nc.vector.tensor_copy(dst[:], ps[:])

        # s_self / s_neigh dot products; F chunks cast to bf16 with a ones
        # column appended (rowsum via matmul)
        s_self_c = const.tile([P, nt], f32)     # [q, t] -> s_self[t*128+q]
        s_neigh_c = const.tile([P, nj], f32)    # [q, c] -> s_neigh[c*128+q]

        a_neigh_bb = const.tile([P, d], bf16)
        nc.vector.tensor_copy(a_neigh_bb[:], a_neigh_b[:])

        for t in range(nt):
            st = pre.tile([P, d], f32, tag="pre_in")
            nc.scalar.dma_start(st[:], self_d[t * P:(t + 1) * P, :])
            scr = pre.tile([P, d], f32, tag="pre_scr")
            nc.vector.scalar_tensor_tensor(
                scr[:], st[:], 1.0, a_self_b[:], Op.mult, Op.mult,
                accum_out=s_self_c[:, t:t + 1])
        for c in range(nj):
            scr = pre.tile([P, d], f32, tag="pre_scr")
            nc.vector.scalar_tensor_tensor(
                scr[:], f_sb[:, c, 0:P], 1.0, a_neigh_bb[:],
                Op.mult, Op.mult,
                accum_out=s_neigh_c[:, c:c + 1])

        # tiny exponentials
        u_c = const.tile([P, nt], f32)
        uh_c = const.tile([P, nt], f32)
        nc.scalar.activation(u_c[:], s_self_c[:], AF.Exp)
        nc.scalar.activation(uh_c[:], s_self_c[:], AF.Exp, scale=0.2)
        v_c = const.tile([P, nj], f32)
        vh_c = const.tile([P, nj], f32)
        nc.scalar.activation(v_c[:], s_neigh_c[:], AF.Exp)
        nc.scalar.activation(vh_c[:], s_neigh_c[:], AF.Exp, scale=0.2)

        # broadcast v / vh along partitions -> [P, m] bf16.
        # Per chunk: PE-transpose one column to a [1,128] psum row, outer-
        # product with ones into a [128,512] psum bank (4 chunks), then one
        # ACT copy psum->sbuf per bank.
        vb = const.tile([P, nj, P], bf16)
        vhb = const.tile([P, nj, P], bf16)
        for src, dst in ((v_c, vb), (vh_c, vhb)):
            for c0 in range(0, nj, 4):
                bank = psum_pre.tile([P, 4 * P], f32, tag="ps_bank")
                for c in range(c0, min(c0 + 4, nj)):
                    pst = psum_pre.tile([1, P], f32, tag="ps_row")
                    nc.tensor.transpose(pst[:], src[:, c:c + 1], ident[:])
                    row = pre.tile([1, P], f32, tag="pre_row")
                    nc.scalar.copy(row[:], pst[:])
                    nc.tensor.matmul(bank[:, (c - c0) * P:(c - c0 + 1) * P],
                                     ones1[:], row[:])
                lo, hi = c0, min(c0 + 4, nj)
                nc.scalar.copy(
                    dst[:, lo:hi, :].rearrange("p c q -> p (c q)"),
                    bank[:, 0:(hi - lo) * P])

        vb_flat = vb[:].rearrange("p c q -> p (c q)")
        vhb_flat = vhb[:].rearrange("p c q -> p (c q)")

        pre_ctx.close()  # release precompute SBUF/PSUM pools

        G = 4 if mm_fstat else 1
        worka = ctx.enter_context(
            tc.tile_pool(name="worka", bufs=2 if mm_fstat else 3))
        workc = ctx.enter_context(
            tc.tile_pool(name="workc", bufs=2 if mm_fstat else 3))
        workp = ctx.enter_context(
            tc.tile_pool(name="workp", bufs=2 if mm_fstat else 3))
        ptp = ctx.enter_context(
            tc.tile_pool(name="ptp", bufs=2 if mm_fstat else 3))
        psum_mm = ctx.enter_context(
            tc.tile_pool(name="psum_mm", bufs=4 if mm_fstat else 6,
                         space="PSUM"))
        outp = ctx.enter_context(tc.tile_pool(name="outp", bufs=3))
        small = ctx.enter_context(tc.tile_pool(name="small", bufs=8))

        def elementwise(rep, t):
            """Produce p_t (masked exp scores) and its reciprocal rowsum."""
            a_t = worka.tile([P, m], bf16, tag="a")
            nc.scalar.mul(a_t[:], vb_flat, u_c[:, t:t + 1])
            c_t = workc.tile([P, m], bf16, tag="c")
            nc.vector.tensor_scalar_mul(c_t[:], vhb_flat, uh_c[:, t:t + 1])
            nc.vector.tensor_tensor(c_t[:], c_t[:], a_t[:], Op.max)
            gi = rep * nt + t
            if gi in mask_tiles:
                mask_t = mask_tiles.pop(gi)
            else:
                mask_t = maskp.tile([P, m], bf16, tag="mask")
                nc.gpsimd.dma_start(mask_t[:], m_d[t * P:(t + 1) * P, :])
            p_t = workp.tile([P, m], bf16, tag="p")
            nc.vector.tensor_tensor(p_t[:], c_t[:], mask_t[:], Op.mult)
            return p_t

        # ---------------- main loop over node tiles ----------------
        if not mm_fstat:
            for rep in range(reps):
                for t in range(nt):
                    p_t = elementwise(rep, t)
                    # blocked transpose: pT[q, c, r] = p[r, c*128+q]
                    pt_t = ptp.tile([P, nj, P], bf16)
                    nc.sync.dma_start(pt_t[:], p_t[:], transpose=True)
                    # psum[128, 129] += pT_c^T @ [F_c | 1]
                    acc = psum_mm.tile([P, d + 1], f32)
                    for c in range(nj):
                        nc.tensor.matmul(acc[:], pt_t[:, c, :], f_sb[:, c, :],
                                         start=(c == 0), stop=(c == nj - 1))
                    rec_t = small.tile([P, 1], f32, tag="rec")
                    nc.vector.reciprocal(rec_t[:], acc[:, d:d + 1])
                    o_t = outp.tile([P, d], f32)
                    nc.vector.tensor_scalar_mul(o_t[:], acc[:, 0:d], rec_t[:])
                    nc.sync.dma_start(out_d[t * P:(t + 1) * P, :], o_t[:])
        else:
            # F-stationary: per group of G node tiles, 32 weight loads and 32
            # wide matmuls (rhs = G tiles' pT chunks side by side); rowsums on
            # GPSIMD; output comes out transposed and is xbar-transposed back
            # in bf16 before the reciprocal scale.
            assert nt % G == 0
            for rep in range(reps):
                for g in range(nt // G):
                    recs = []
                    ptg = ptp.tile([P, nj, G, P], bf16, tag="ptg")
                    for ti in range(G):
                        t = g * G + ti
                        p_t = elementwise(rep, t)
                        rs_t = small.tile([P, 1], f32, tag="rs")
                        nc.gpsimd.reduce_sum(rs_t[:], p_t[:],
                                             axis=mybir.AxisListType.X)
                        rec_t = small.tile([P, 1], f32, tag="rec")
                        nc.vector.reciprocal(rec_t[:], rs_t[:])
                        recs.append(rec_t)
                        nc.sync.dma_start(ptg[:, :, ti, :], p_t[:],
                                          transpose=True)
                    accT = psum_mm.tile([P, G * P], f32)
                    for c in range(nj):
                        nc.tensor.matmul(
                            accT[:], f_sb[:, c, 0:P],
                            ptg[:, c, :, :].rearrange("p g q -> p (g q)"),
                            start=(c == 0), stop=(c == nj - 1))
                    outT = outp.tile([P, G * P], bf16, tag="outT")
                    nc.scalar.copy(outT[:], accT[:])
                    o4 = outp.tile([P, G, P], bf16, tag="o4")
                    nc.sync.dma_start(o4[:], outT[:], transpose=True)
                    for ti in range(G):
                        t = g * G + ti
                        o_t = outp.tile([P, d], f32, tag="of")
                        nc.vector.tensor_scalar_mul(o_t[:], o4[:, ti, :],
                                                    recs[ti][:])
                        nc.sync.dma_start(out_d[t * P:(t + 1) * P, :], o_t[:])

    nc.compile()
    return nc


def _get_nc(key=(NLOC, M, D)):
    if key not in _BUILT:
        _BUILT[key] = _build_nc(*key)
    return _BUILT[key]


def kernel(self_feats, features_neighs, neigh_matrix, a):
    from concourse.bass_utils import run_bass_kernel_spmd

    self_feats = np.ascontiguousarray(self_feats, dtype=np.float32)
    features_neighs = np.ascontiguousarray(features_neighs, dtype=np.float32)
    neigh_matrix = np.ascontiguousarray(neigh_matrix, dtype=np.int32)
    a = np.ascontiguousarray(a, dtype=np.float32)

    nc = _get_nc()
    in_maps = []
    for c in range(NCORES):
        sl = slice(c * NLOC, (c + 1) * NLOC)
        in_maps.append({
            "self_feats": self_feats[sl],
            "features_neighs": features_neighs,
            "neigh_matrix": neigh_matrix[sl],
            "a": a,
        })
    res = run_bass_kernel_spmd(nc, in_maps, core_ids=list(range(NCORES)))
    out = np.concatenate([np.asarray(res.results[c]["out"])
                          for c in range(NCORES)], axis=0)
    return out.astype(np.float32)
